# revision 1
# baseline (speedup 1.0000x reference)
"""Trainium2 Bass kernel for a GPT-style transformer block.

Problem: nn_Block_36807869727037 (dense_transformer)
  B=4, T=2048, C=1024, H=16 heads (d=64), fp32 I/O.
  y = x + attn(LN1(x)); y = y + mlp(LN2(y))  (causal attention, tanh-GELU MLP)

Sharding (8 cores, one uniform SPMD program):
  core = 2*b + s  -> batch b in [0,4), tensor-parallel shard s in [0,2).
  Shard s owns heads [8s, 8s+8) and FFN hidden slice [2048s, 2048s+2048).
  Final combine on host: out[b] = x2 (from core 2b) + mlp_partial(2b)
                                  + mlp_partial(2b+1) + b_mlp_proj.

v2 performance structure (vs the 880us baseline; ~700us measured):
  - One ACT table set per phase: LN inv-std via DVE Quake-rsqrt + 2 Newton
    steps, so Scalar runs only Exp in attention and Gelu in the MLP (the
    baseline paid 40 ACT_TABLE_LOADs at ~2.7us ping-ponging exp/sqrt/gelu).
    The gelu prewarm is dependency-pinned so it can't float into attention.
  - Matmuls with half-size stationaries (K=64 scores / M=65 PV / K<=2
    denominator broadcast) stream at HALF the PE rate, so all are padded
    to full 128x128: Q^T zero-padded by head parity, PV via a 128-wide V
    window (spill rows of po unread), denominator via zero-padded
    selectors against a persistent zeroed staging row.
  - Attention head-pair interleave at key-tile granularity with +1 score
    prefetch; V/QK of run r+1 and c_proj of run r-1 are emitted as filler
    units inside the head-loop iterations so TensorE never drains while
    the scalar engine works through the exps (also keeps the HAM clock
    gate at 2.4GHz; baseline PV matmuls ran at the 1.2GHz cold rate).
  - Diagonal score/PV matmuls narrowed to the causal column range.
  - xnT/xn2T built by DMA-transpose (sync queue), not TensorE matmuls.
  - b_proj/2 folded into the c_proj PSUM->SBUF cast on each pair core.
  - wf streamed during run 3 (wearly/xnT pools retire after run 2, wf
    rides the right-side ring); wm during MLP quarter 0; startup x loads
    jump ahead of the descriptor-heavy broadcast-const DMAs.
"""

import os
import sys

sys.path.insert(0, "/opt/trn_rl_repo")

import numpy as np
import ml_dtypes

B, T, C, H = 4, 2048, 1024, 16
D = C // H          # 64 head dim
HPC = H // 2        # 8 heads per core
FPC = 2 * C         # 2048 ffn hidden per core
P = 128
EPS = 1e-10
NT = T // P         # 16 token tiles
NR = T // 512       # 4 query runs of 512
CK = C // P         # 8 feature chunks
FK = FPC // P       # 16 ffn chunks per core
EK = (HPC * D) // P  # 4 head-dim chunks per core (512/128)

RSQRT_MAGIC = 0x5F3759DF

_CACHED = {}


def _build_nc():
    import concourse.bass as bass
    import concourse.mybir as mybir
    import concourse.tile as tile
    from concourse import bacc

    f32 = mybir.dt.float32
    i32 = mybir.dt.int32
    bf16 = mybir.dt.bfloat16
    AF = mybir.ActivationFunctionType
    ALU = mybir.AluOpType

    nc = bacc.Bacc(trn_type="TRN2", target_bir_lowering=False, num_devices=8)

    # ---- I/O ----
    x_d = nc.dram_tensor("x", [T, C], f32, kind="ExternalInput")
    wqT_d = nc.dram_tensor("wqT", [C, HPC * D], bf16, kind="ExternalInput")
    wkT_d = nc.dram_tensor("wkT", [C, HPC * D], bf16, kind="ExternalInput")
    wvT_d = nc.dram_tensor("wvT", [C, HPC * D], bf16, kind="ExternalInput")
    wpT_d = nc.dram_tensor("wpT", [HPC * D, C], bf16, kind="ExternalInput")
    wfT_d = nc.dram_tensor("wfT", [C, FPC], bf16, kind="ExternalInput")
    wmT_d = nc.dram_tensor("wmT", [FPC, C], bf16, kind="ExternalInput")
    bqk_d = nc.dram_tensor("bqk", [P, 2 * EK], f32, kind="ExternalInput")
    bv_d = nc.dram_tensor("bv", [HPC * D], bf16, kind="ExternalInput")
    bprojh_d = nc.dram_tensor("bprojh", [C], bf16, kind="ExternalInput")
    bfc_d = nc.dram_tensor("bfc", [P, FK], f32, kind="ExternalInput")
    mask_d = nc.dram_tensor("mask", [P, P], bf16, kind="ExternalInput")

    out_mlp_d = nc.dram_tensor("out_mlp", [T, C], f32, kind="ExternalOutput")
    out_x2_d = nc.dram_tensor("out_x2", [T, C], f32, kind="ExternalOutput")

    cc_in_d = nc.dram_tensor("cc_in", [T, C], bf16)
    cc_out_d = nc.dram_tensor("cc_out", [T, C], bf16)

    def bcast_row(dram_ap, n):
        # [n] DRAM vector -> [P, n] broadcast AP (partition-step 0)
        return bass.AP(
            tensor=dram_ap.tensor, offset=dram_ap.offset,
            ap=[[0, P], *dram_ap.ap],
        )

    def bcast_part(sb_ap, npart):
        # [1, n] SBUF AP -> [npart, n] partition-broadcast AP
        return bass.AP(
            tensor=sb_ap.tensor, offset=sb_ap.offset,
            ap=[[0, npart], *sb_ap.ap[1:]],
        )

    with tile.TileContext(nc, pool_alloc_mode="queue") as tc:
        import contextlib

        with contextlib.ExitStack() as ctx:
            consts = ctx.enter_context(tc.tile_pool(name="consts", bufs=1))
            work = ctx.enter_context(tc.tile_pool(name="work", bufs=3))
            co_pool = ctx.enter_context(tc.tile_pool(name="co", bufs=2))
            ln_pool = ctx.enter_context(tc.tile_pool(name="ln", bufs=2))
            small = ctx.enter_context(tc.tile_pool(name="small", bufs=1))
            den_pool = ctx.enter_context(tc.tile_pool(name="den", bufs=2))
            x_pool = ctx.enter_context(tc.tile_pool(name="xin", bufs=4))
            x2_pool = ctx.enter_context(tc.tile_pool(name="x2", bufs=4))
            ppool = ctx.enter_context(tc.tile_pool(name="psum", bufs=2, space="PSUM"))
            scpool = ctx.enter_context(
                tc.tile_pool(name="psum_sc", bufs=4, space="PSUM"))
            pvpool = ctx.enter_context(
                tc.tile_pool(name="psum_pv", bufs=1, space="PSUM"))

            eps_sb = consts.tile([P, 1], f32)
            nc.vector.memset(eps_sb[:], EPS)
            shift1_sb = consts.tile([P, 1], i32)
            nc.vector.memset(shift1_sb[:], 1)
            neg1_sb = consts.tile([P, 1], i32)
            nc.vector.memset(neg1_sb[:], -1)
            magic_sb = consts.tile([P, 4], i32)
            nc.vector.memset(magic_sb[:], RSQRT_MAGIC + 1)
            scratch1 = consts.tile([P, 1], f32)
            # prewarm the exp table set while the initial DMAs run
            nc.scalar.activation(
                out=scratch1[:], in_=eps_sb[:], func=AF.Exp, scale=1.0)
            # selectors for the denominator partition-broadcast matmuls:
            # pb2 = selA^T recA + selB^T recB gives rows 0-63 = recA,
            # rows 64-127 = recB. Zero-padded to K=128 (with a persistent
            # zeroed rec staging row) so they run at the full PE rate.
            selA_sb = consts.tile([P, P], bf16)
            nc.gpsimd.memset(selA_sb[:], 0.0)
            nc.gpsimd.memset(selA_sb[0:1, 0:D], 1.0)
            selB_sb = consts.tile([P, P], bf16)
            nc.gpsimd.memset(selB_sb[:], 0.0)
            nc.gpsimd.memset(selB_sb[0:1, D : 2 * D], 1.0)
            recbf_sb = consts.tile([P, 1024], bf16)
            nc.gpsimd.memset(recbf_sb[:], 0.0)

            def load_x(tt, engine):
                x_sb = x_pool.tile([P, C], f32, tag="x")
                engine.dma_start(x_sb[:], x_d[tt * P : (tt + 1) * P, :])
                return x_sb

            # run-0 x tiles FIRST on the sync HW queue (the broadcast-const
            # loads are descriptor-heavy and would delay LN1 by ~15us);
            # weights on the scalar queue in consumption order (Q/K first)
            x_run0 = [load_x(tt, nc.sync) for tt in range(4)]
            bqk_sb = consts.tile([P, 2 * EK], f32)
            nc.sync.dma_start(bqk_sb[:], bqk_d[:])
            mask_sb = consts.tile([P, P], bf16)
            nc.sync.dma_start(mask_sb[:], mask_d[:])
            bfc_sb = consts.tile([P, FK], f32)
            nc.sync.dma_start(bfc_sb[:], bfc_d[:])
            bv_sb = consts.tile([P, HPC * D], bf16)
            nc.sync.dma_start(bv_sb[:], bcast_row(bv_d[:], HPC * D))
            bprojh_sb = consts.tile([P, C], bf16)
            nc.sync.dma_start(bprojh_sb[:], bcast_row(bprojh_d[:], C))

            # pool stacking note: wearly/p_xnT are opened LAST on the left
            # stack so they can be released right after run 2 (their final
            # readers are run-3's V/QK fillers, which run during run 2) —
            # freeing ring space while wp/attn/qt/pt live on.
            wp_cm = tc.tile_pool(name="wp", bufs=1)
            wp_pool = wp_cm.__enter__()
            wp_sb = wp_pool.tile([P, EK, C], bf16)

            # ---- persistent attention state ----
            attn_cm = tc.tile_pool(name="attn", bufs=1)
            attn_pool = attn_cm.__enter__()
            # Half-size matmul operands (K=64 scores / M=65 PV) stream at
            # half the PE rate, so pad both to full 128x128 stationaries:
            # Q^T is stored zero-padded by head parity (cross-head products
            # vanish against the zeros), and PV uses a 128-wide V window
            # whose extra columns land in unread po rows.
            KT = attn_pool.tile([P, EK, T], bf16)
            VW = HPC * (D + 1)
            V_aug = attn_pool.tile([P, NT, VW + 63], bf16)
            nc.gpsimd.memset(V_aug[:, :, VW : VW + 63], 0.0)
            V_hv = V_aug[:, :, 0:VW].rearrange("p n (h e) -> p n h e", e=D + 1)
            nc.gpsimd.memset(V_hv[:, :, :, D : D + 1], 1.0)
            OT = attn_pool.tile([P, EK, 512], bf16)

            qt_cm = tc.tile_pool(name="p_qt", bufs=2)
            qt_pool = qt_cm.__enter__()

            pt_cm = tc.tile_pool(name="ptp", bufs=6)
            pt_pool = pt_cm.__enter__()

            wearly_cm = tc.tile_pool(name="wearly", bufs=1)
            wearly = wearly_cm.__enter__()
            wq_sb = wearly.tile([P, CK, HPC * D], bf16)
            nc.scalar.dma_start(
                wq_sb[:], wqT_d.ap().rearrange("(k p) o -> p k o", p=P))
            wk_sb = wearly.tile([P, CK, HPC * D], bf16)
            nc.scalar.dma_start(
                wk_sb[:], wkT_d.ap().rearrange("(k p) o -> p k o", p=P))
            wv_sb = wearly.tile([P, CK, HPC * D], bf16)
            nc.scalar.dma_start(
                wv_sb[:], wvT_d.ap().rearrange("(k p) o -> p k o", p=P))
            nc.scalar.dma_start(
                wp_sb[:], wpT_d.ap().rearrange("(k p) o -> p k o", p=P))

            xnT_cm = tc.tile_pool(name="p_xnT", bufs=2)
            p_xnT = xnT_cm.__enter__()

            xn2T_cm = tc.tile_pool(name="p_xn2T", bufs=1, side="right")
            p_xn2T = xn2T_cm.__enter__()
            xn2T = p_xn2T.tile([P, CK, T], bf16)

            def rsqrt_dve(out_f32, v_f32, tmp_i32, n):
                # out = 1/sqrt(v) elementwise: Quake seed + 2 Newton steps.
                vb = v_f32.bitcast(i32)
                nc.vector.tensor_scalar(
                    out=tmp_i32, in0=vb, scalar1=shift1_sb[:, 0:1],
                    scalar2=neg1_sb[:, 0:1],
                    op0=ALU.logical_shift_right, op1=ALU.bitwise_xor,
                )
                nc.vector.tensor_add(
                    out=tmp_i32, in0=tmp_i32, in1=magic_sb[:, 0:n])
                y = tmp_i32.bitcast(f32)
                for _ in range(2):
                    nc.vector.tensor_mul(out=out_f32, in0=y, in1=y)
                    nc.vector.tensor_mul(out=out_f32, in0=out_f32, in1=v_f32)
                    nc.vector.tensor_scalar(
                        out=out_f32, in0=out_f32, scalar1=-0.5, scalar2=1.5,
                        op0=ALU.mult, op1=ALU.add,
                    )
                    nc.vector.tensor_mul(out=y, in0=y, in1=out_f32)
                nc.vector.tensor_copy(out=out_f32, in_=y)

            def ln_batch(x_sbs, xnT_dst):
                # LayerNorm n token tiles -> bf16, feature-major via
                # DMA-transpose into xnT_dst[:, :, i*P:(i+1)*P].
                n = len(x_sbs)
                stats = ln_pool.tile([P, n, 2, 6], f32, tag=f"ln_stats{n}")
                for i, x_sb in enumerate(x_sbs):
                    xg = x_sb[:].rearrange("p (g f) -> p g f", f=512)
                    for g in range(2):
                        nc.vector.bn_stats(out=stats[:, i, g, :], in_=xg[:, g, :])
                mv4 = ln_pool.tile([P, n, 2], f32, tag=f"ln_mv{n}")
                for i in range(n):
                    nc.vector.bn_aggr(out=mv4[:, i, :], in_=stats[:, i, :, :])
                v4 = ln_pool.tile([P, n], f32, tag=f"ln_v{n}")
                nc.vector.tensor_scalar_add(
                    out=v4[:], in0=mv4[:, :, 1], scalar1=eps_sb[:, 0:1])
                rs4 = ln_pool.tile([P, n], f32, tag=f"ln_rs{n}")
                t_i = ln_pool.tile([P, n], i32, tag=f"ln_ti{n}")
                rsqrt_dve(rs4[:], v4[:], t_i[:], n)
                for i, x_sb in enumerate(x_sbs):
                    xn_bf = work.tile([P, C], bf16, tag="bf16buf")
                    nc.vector.tensor_scalar(
                        out=xn_bf[:], in0=x_sb[:],
                        scalar1=mv4[:, i, 0:1], scalar2=rs4[:, i : i + 1],
                        op0=ALU.subtract, op1=ALU.mult,
                    )
                    nc.sync.dma_start_transpose(
                        xnT_dst[:, :, i * P : (i + 1) * P], xn_bf[:])

            def emit_v_tile(xnT_r, i, tt):
                ps = ppool.tile([P, 512], f32, tag="mm")
                for ck in range(CK):
                    nc.tensor.matmul(
                        ps[:],
                        xnT_r[:, ck, i * P : (i + 1) * P],
                        wv_sb[:, ck, :],
                        start=(ck == 0), stop=(ck == CK - 1),
                    )
                nc.vector.tensor_add(
                    out=V_hv[:, tt, :, 0:D],
                    in0=ps[:].rearrange("p (h e) -> p h e", h=HPC),
                    in1=bv_sb[:].rearrange("p (h e) -> p h e", h=HPC),
                )

            def emit_qk_one(xnT_r, r, QT_dst, ot):
                # one output tile of Q^T (ot<EK) or K^T (ot>=EK) for run r
                w_sb = wq_sb if ot < EK else wk_sb
                ol = (ot % EK) * P
                ps = ppool.tile([P, 512], f32, tag="mm")
                for ck in range(CK):
                    nc.tensor.matmul(
                        ps[:],
                        w_sb[:, ck, ol : ol + P],
                        xnT_r[:, ck, :],
                        start=(ck == 0), stop=(ck == CK - 1),
                    )
                if ot < EK:
                    nc.vector.tensor_scalar_add(
                        out=QT_dst[0:D, 0, ot, :], in0=ps[0:D, :],
                        scalar1=bqk_sb[0:D, ot : ot + 1],
                    )
                    nc.vector.tensor_scalar_add(
                        out=QT_dst[D : 2 * D, 1, ot, :],
                        in0=ps[D : 2 * D, :],
                        scalar1=bqk_sb[D : 2 * D, ot : ot + 1],
                    )
                else:
                    nc.vector.tensor_scalar_add(
                        out=KT[:, ot % EK, r * 512 : (r + 1) * 512],
                        in0=ps[:], scalar1=bqk_sb[:, ot : ot + 1],
                    )

            def emit_cproj_tile(r, i):
                # c_proj partial for token tile 4r+i + pair-AllReduce trigger
                tt = 4 * r + i
                cc_sb = co_pool.tile([P, C], bf16, tag="ccbuf")
                for half in range(2):
                    ps = ppool.tile([P, 512], f32, tag="mm")
                    for ek in range(EK):
                        nc.tensor.matmul(
                            ps[:],
                            OT[:, ek, i * P : (i + 1) * P],
                            wp_sb[:, ek, half * 512 : (half + 1) * 512],
                            start=(ek == 0), stop=(ek == EK - 1),
                        )
                    nc.vector.tensor_add(
                        out=cc_sb[:, half * 512 : (half + 1) * 512],
                        in0=ps[:],
                        in1=bprojh_sb[:, half * 512 : (half + 1) * 512],
                    )
                nc.gpsimd.dma_start(
                    cc_in_d[tt * P : (tt + 1) * P, :], cc_sb[:])
                if i == 3:
                    nc.gpsimd.collective_compute(
                        "AllReduce",
                        ALU.add,
                        replica_groups=[[0, 1], [2, 3], [4, 5], [6, 7]],
                        ins=[cc_in_d[r * 512 : (r + 1) * 512, :].opt()],
                        outs=[cc_out_d[r * 512 : (r + 1) * 512, :].opt()],
                    )

            def emit_x2_tile(tt):
                # x2 = x + (attn + b_proj) [cc_out]; store out_x2 (cast f32)
                x_sb = load_x(tt, nc.gpsimd)
                att_sb = work.tile([P, C], bf16, tag="bf16buf")
                nc.gpsimd.dma_start(
                    att_sb[:], cc_out_d[tt * P : (tt + 1) * P, :])
                x2_sb = x2_pool.tile([P, C], bf16, tag="x2")
                nc.gpsimd.tensor_add(out=x2_sb[:], in0=x_sb[:], in1=att_sb[:])
                nc.gpsimd.dma_start(
                    out_x2_d[tt * P : (tt + 1) * P, :], x2_sb[:])
                return x2_sb

            # ======== run 0 preamble ========
            xnT_r = p_xnT.tile([P, CK, 512], bf16, tag="xnT")
            ln_batch(x_run0, xnT_r)
            qt_first = [2]

            def new_qt():
                # the zero parity-pads survive buffer rotation (only the
                # valid halves are rewritten), so memset just the first use
                # of each of the 2 pool buffers
                qt_tile = qt_pool.tile([P, 2, EK, 512], bf16, tag="QT")
                if qt_first[0] > 0:
                    qt_first[0] -= 1
                    nc.gpsimd.memset(qt_tile[D : 2 * D, 0, :, :], 0.0)
                    nc.gpsimd.memset(qt_tile[0:D, 1, :, :], 0.0)
                return qt_tile

            QT = new_qt()
            # emit Q/K output tiles pair-interleaved (Q0,K0,Q1,K1,...) so
            # head pair hc only waits on the 2 chains it actually reads
            QK_ORDER = [o for i in range(EK) for o in (i, EK + i)]
            for ot in QK_ORDER:
                emit_qk_one(xnT_r, 0, QT, ot)
            for i in range(4):
                emit_v_tile(xnT_r, i, i)

            # ======== fused pipeline over the 4 query runs ========
            for r in range(NR):
                ns = 4 * r + 4
                xnT_next = None
                QT_next = None
                x_next = None
                if r < NR - 1:
                    x_next = [load_x(tt, nc.gpsimd)
                              for tt in range(4 * (r + 1), 4 * (r + 1) + 4)]

                # filler units: dense matmul chains slotted into the head
                # loop's st iterations so TensorE never drains while the
                # scalar engine works through the exps
                fillers = []
                if r >= 1:
                    fillers += [
                        (lambda i=i, rr=r - 1: emit_cproj_tile(rr, i))
                        for i in range(4)
                    ]

                def emit_tail_rec(poA, poB):
                    # stage 1 (right after the pair's last PV): reciprocal
                    # of the two denominators into the zero-padded staging row
                    d2 = small.tile([1, 1024], f32, tag="d2")
                    nc.vector.tensor_copy(
                        out=d2[0:1, 0:512], in_=poA[D : D + 1, :])
                    nc.vector.tensor_copy(
                        out=d2[0:1, 512:1024], in_=poB[D : D + 1, :])
                    rec2 = small.tile([1, 1024], f32, tag="rec2")
                    nc.vector.reciprocal_approx_fast(out=rec2[:], in_=d2[:])
                    nc.vector.tensor_copy(out=recbf_sb[0:1, :], in_=rec2[:])

                def emit_tail(hc, poA, poB):
                    # stage 2: broadcast across partitions and normalize
                    pb2 = ppool.tile([P, 512], f32, tag="mm")
                    nc.tensor.matmul(
                        pb2[:], selA_sb[:], recbf_sb[:, 0:512],
                        start=True, stop=False)
                    nc.tensor.matmul(
                        pb2[:], selB_sb[:], recbf_sb[:, 512:1024],
                        start=False, stop=True)
                    den = den_pool.tile([P, 512], f32, tag="den")
                    nc.vector.tensor_copy(out=den[:], in_=pb2[:])
                    nc.vector.tensor_mul(
                        out=OT[0:D, hc, :], in0=poA[0:D, :], in1=den[0:D, :])
                    nc.vector.tensor_mul(
                        out=OT[D : 2 * D, hc, :], in0=poB[0:D, :],
                        in1=den[D : 2 * D, :])

                pending_tail = None
                for hc in range(HPC // 2):
                    hA, hB = 2 * hc, 2 * hc + 1
                    poA = pvpool.tile([P, 512], f32, tag="poA")
                    poB = pvpool.tile([P, 512], f32, tag="poB")

                    def emit_s(h, st):
                        # scores S^T[key tile st, 512 queries] for head h:
                        # full 128-row stationary; the zero-padded parity
                        # copy of K^T cancels the other head's Q rows
                        j = st - 4 * r
                        lo = max(j, 0) * P
                        sc = scpool.tile([P, 512], f32, tag="sc")
                        nc.tensor.matmul(
                            sc[:, lo:512],
                            KT[:, hc, st * P : (st + 1) * P],
                            QT[:, h % 2, hc, lo:512],
                            start=True, stop=True,
                        )
                        return sc

                    def emit_exp(h, st, sc):
                        j = st - 4 * r
                        lo = max(j, 0) * P
                        PT = pt_pool.tile([P, 512], bf16, tag="PT")
                        nc.scalar.activation(
                            out=PT[:, lo:512], in_=sc[:, lo:512],
                            func=AF.Exp, scale=0.125)
                        if j >= 0:
                            nc.vector.tensor_mul(
                                out=PT[:, lo : lo + P],
                                in0=PT[:, lo : lo + P],
                                in1=mask_sb[:],
                            )
                        return PT

                    def emit_pv(h, st, PT, po):
                        # 128-wide stationary window: po rows 65-127 get the
                        # next head's garbage and are never read
                        j = st - 4 * r
                        lo = max(j, 0) * P
                        c0 = h * (D + 1)
                        nc.tensor.matmul(
                            po[:, lo:512],
                            V_aug[:, st, c0 : c0 + P],
                            PT[:, lo:512],
                            start=(st == 0), stop=(st == ns - 1),
                        )

                    scA = emit_s(hA, 0)
                    scB = emit_s(hB, 0)
                    for st in range(ns):
                        scA_n = emit_s(hA, st + 1) if st + 1 < ns else None
                        scB_n = emit_s(hB, st + 1) if st + 1 < ns else None
                        if st == 0 and pending_tail is not None:
                            emit_tail(*pending_tail)
                            pending_tail = None
                        ptA = emit_exp(hA, st, scA)
                        ptB = emit_exp(hB, st, scB)
                        if fillers:
                            fillers.pop(0)()
                        emit_pv(hA, st, ptA, poA)
                        emit_pv(hB, st, ptB, poB)
                        scA, scB = scA_n, scB_n
                    emit_tail_rec(poA, poB)
                    pending_tail = (hc, poA, poB)

                    # side work (after this pair's tail next iteration):
                    # LN1 of next run after pair 1, x2 of prev run after
                    # pair 2; V/QK of next run queue as fillers at pair 2
                    if hc == 1 and r < NR - 1:
                        xnT_next = p_xnT.tile([P, CK, 512], bf16, tag="xnT")
                        ln_batch(x_next, xnT_next)
                    if hc == 2:
                        if r >= 1:
                            x2s = [emit_x2_tile(tt)
                                   for tt in range(4 * (r - 1), 4 * (r - 1) + 4)]
                            ln_batch(x2s, xn2T[:, :, (r - 1) * 512 : r * 512])
                        if r < NR - 1:
                            QT_next = new_qt()
                            fillers += [
                                (lambda i=i: emit_v_tile(
                                    xnT_next, i, 4 * (r + 1) + i))
                                for i in range(4)
                            ]
                            fillers += [
                                (lambda ot=ot: emit_qk_one(
                                    xnT_next, r + 1, QT_next, ot))
                                for ot in QK_ORDER
                            ]

                # drain leftover fillers, then the last pair's tail
                for f in fillers:
                    f()
                fillers = []
                emit_tail(*pending_tail)
                pending_tail = None
                if r < NR - 1:
                    xnT_r = xnT_next
                    QT = QT_next
                else:
                    # last run: c_proj has no following head loop to hide in
                    for i in range(4):
                        emit_cproj_tile(r, i)
                if r == NR - 2:
                    # Q/K/V weights and xnT are dead once run-3's V/QK
                    # fillers have been emitted — release them and stream
                    # the fc weight in during run 3 (right-side stack)
                    xnT_cm.__exit__(None, None, None)
                    wearly_cm.__exit__(None, None, None)
                    wf_cm = tc.tile_pool(name="p_wf", bufs=1, side="right")
                    wf_pool = wf_cm.__enter__()
                    wf_sb = wf_pool.tile([P, CK, FPC], bf16)
                    wf_src = wfT_d.ap().rearrange("(k p) o -> p k o", p=P)
                    for c8 in range(8):
                        sl = slice(c8 * 256, (c8 + 1) * 256)
                        eng = nc.sync if c8 % 2 == 0 else nc.scalar
                        eng.dma_start(wf_sb[:, :, sl], wf_src[:, :, sl])

            # release attention-phase SBUF before the MLP phase
            pt_cm.__exit__(None, None, None)
            qt_cm.__exit__(None, None, None)
            attn_cm.__exit__(None, None, None)
            wp_cm.__exit__(None, None, None)

            with tc.tile_pool(name="wlate", bufs=1) as wlate, \
                 tc.tile_pool(name="p_hT", bufs=1, side="right") as p_hT:
                wm_sb = wlate.tile([P, FK, C], bf16)
                nc.gpsimd.dma_start(
                    wm_sb[:], wmT_d.ap().rearrange("(k p) o -> p k o", p=P))

                # prewarm the gelu table while ACT is idle; input depends on
                # the last attention tail so the scheduler can't float this
                # above the exps (two extra table switches if it does)
                nc.scalar.activation(
                    out=scratch1[0:1, 0:1], in_=recbf_sb[0:1, 0:1],
                    func=AF.Gelu_apprx_tanh, bias=0.0, scale=1.0)

                # ======== MLP in 4 token quarters ========
                for tq in range(4):
                    if tq == 2:
                        x2s = [emit_x2_tile(tt) for tt in range(12, 16)]
                        ln_batch(x2s, xn2T[:, :, 3 * 512 : 4 * 512])
                    t0 = tq * 512
                    hT = p_hT.tile([P, FK, 512], bf16, tag="hT")
                    for ft in range(FK):
                        ps = ppool.tile([P, 512], f32, tag="mm")
                        for ck in range(CK):
                            nc.tensor.matmul(
                                ps[:],
                                wf_sb[:, ck, ft * P : (ft + 1) * P],
                                xn2T[:, ck, t0 : t0 + 512],
                                start=(ck == 0), stop=(ck == CK - 1),
                            )
                        nc.scalar.activation(
                            out=hT[:, ft, :], in_=ps[:],
                            func=AF.Gelu_apprx_tanh,
                            bias=bfc_sb[:, ft : ft + 1], scale=1.0,
                        )
                    for tl in range(4):
                        out_sb = co_pool.tile([P, C], f32, tag="obuf")
                        for half in range(2):
                            ps = ppool.tile([P, 512], f32, tag="mm")
                            for fk in range(FK):
                                nc.tensor.matmul(
                                    ps[:],
                                    hT[:, fk, tl * P : (tl + 1) * P],
                                    wm_sb[:, fk, half * 512 : (half + 1) * 512],
                                    start=(fk == 0), stop=(fk == FK - 1),
                                )
                            nc.vector.tensor_copy(
                                out=out_sb[:, half * 512 : (half + 1) * 512],
                                in_=ps[:],
                            )
                        eng = nc.sync if tl % 2 == 0 else nc.scalar
                        eng.dma_start(
                            out_mlp_d[t0 + tl * P : t0 + (tl + 1) * P, :],
                            out_sb[:],
                        )

            wf_cm.__exit__(None, None, None)
            xn2T_cm.__exit__(None, None, None)

    nc.finalize()
    return nc


def _prep_inputs(x, w_attn, b_attn, w_proj, b_proj, w_fc, b_fc, w_mlp_proj):
    bf = ml_dtypes.bfloat16
    mask = np.triu(np.ones((P, P), dtype=np.float32)).astype(bf)
    in_maps = []
    for core in range(8):
        b, s = divmod(core, 2)
        wq = np.ascontiguousarray(w_attn[s * 512 : (s + 1) * 512, :].T).astype(bf)
        wk = np.ascontiguousarray(
            w_attn[C + s * 512 : C + (s + 1) * 512, :].T).astype(bf)
        wv = np.ascontiguousarray(
            w_attn[2 * C + s * 512 : 2 * C + (s + 1) * 512, :].T).astype(bf)
        bq = b_attn[s * 512 : (s + 1) * 512]
        bk = b_attn[C + s * 512 : C + (s + 1) * 512]
        bv = b_attn[2 * C + s * 512 : 2 * C + (s + 1) * 512]
        bqk = np.concatenate(
            [bq.reshape(EK, P).T, bk.reshape(EK, P).T], axis=1
        ).astype(np.float32)
        wp = np.ascontiguousarray(w_proj[:, s * 512 : (s + 1) * 512].T).astype(bf)
        wf = np.ascontiguousarray(w_fc[s * FPC : (s + 1) * FPC, :].T).astype(bf)
        bfc = np.ascontiguousarray(
            b_fc[s * FPC : (s + 1) * FPC].reshape(FK, P).T).astype(np.float32)
        wm = np.ascontiguousarray(
            w_mlp_proj[:, s * FPC : (s + 1) * FPC].T).astype(bf)
        in_maps.append(
            {
                "x": np.ascontiguousarray(x[b]),
                "wqT": wq, "wkT": wk, "wvT": wv, "wpT": wp, "wfT": wf, "wmT": wm,
                "bqk": bqk, "bv": np.ascontiguousarray(bv).astype(bf),
                "bprojh": (0.5 * np.ascontiguousarray(b_proj)).astype(bf),
                "bfc": bfc, "mask": mask,
            }
        )
    return in_maps


def run(x, w_attn, b_attn, w_proj, b_proj, w_fc, b_fc, w_mlp_proj, b_mlp_proj,
        trace=False):
    from concourse.bass_utils import run_bass_kernel_spmd

    if "nc" not in _CACHED:
        _CACHED["nc"] = _build_nc()
    nc = _CACHED["nc"]
    in_maps = _prep_inputs(
        x, w_attn, b_attn, w_proj, b_proj, w_fc, b_fc, w_mlp_proj
    )
    res = run_bass_kernel_spmd(
        nc, in_maps, core_ids=list(range(8)), trace=trace,
        trace_cores=list(range(8)) if trace else None,
    )
    out = np.empty((B, T, C), dtype=np.float32)
    for b in range(B):
        a = res.results[2 * b]
        c2 = res.results[2 * b + 1]
        out[b] = a["out_x2"] + a["out_mlp"] + c2["out_mlp"] + b_mlp_proj[None, :]
    return out, res


def kernel(x, w_attn, b_attn, w_proj, b_proj, w_fc, b_fc, w_mlp_proj, b_mlp_proj):
    out, _ = run(
        np.asarray(x, dtype=np.float32),
        np.asarray(w_attn, dtype=np.float32),
        np.asarray(b_attn, dtype=np.float32),
        np.asarray(w_proj, dtype=np.float32),
        np.asarray(b_proj, dtype=np.float32),
        np.asarray(w_fc, dtype=np.float32),
        np.asarray(b_fc, dtype=np.float32),
        np.asarray(w_mlp_proj, dtype=np.float32),
        np.asarray(b_mlp_proj, dtype=np.float32),
    )
    return out

